# revision 15
# baseline (speedup 1.0000x reference)
"""Trainium2 Bass kernel for gated attention (nn_Attention_57475252355505).

Reference computation (per batch b):
    q = (q_x @ Wq.T) * 1/sqrt(32), split into H=8 heads of D=32
    k = kv_x @ Wk.T ; v = kv_x @ Wv.T
    a = softmax(q @ k.T + bias)           # bias broadcast over heads
    o = (a @ v) * sigmoid(q_x @ Wg.T + bg)
    out = o @ Wo.T + bo

Sharding: 8 cores, core c handles batch b = c//4 and query rows
[512*(c%4), 512*(c%4+1)).  kv_x/weights are replicated per batch group;
bias/q_x/output are disjoint.  No collectives needed.

Dataflow on each core is in "transposed space" ([feature, token] layouts)
so that every matmul contraction sits on the partition axis:
  - scores are computed as S^T [k, q] so softmax-over-k can use the
    matmul ones-trick for denominators, and the o-matmul needs no
    transposition of the (huge) attention-weight matrix.
  - bias^T is produced once with TensorE identity-matmuls and injected
    into the scores PSUM accumulation (so no elementwise bias pass).
  - the D=32 contractions are packed 4-per-PE-array with tile_position.
"""

import sys

sys.path.insert(0, "/opt/trn_rl_repo")

import numpy as np

import concourse.bass as bass
import concourse.mybir as mybir
import concourse.tile as tile_mod
from concourse.bass_utils import run_bass_kernel_spmd

# ---------------------------------------------------------------------------
# Problem constants (hardcoded per the harness contract).
B, Q, K, C, H, D = 2, 2048, 2048, 256, 8, 32
N_CORES = 8
QS = Q * B // N_CORES  # 512 query rows per core
SCALE = 1.0 / np.sqrt(np.float32(D))

FP32 = mybir.dt.float32
BF16 = mybir.dt.bfloat16

# ---------------------------------------------------------------------------
# This walrus build only accepts a single sync-wait per instruction; Tile's
# semaphore assignment batches several.  After tracing, hoist extra waits
# onto single-wait NOPs on the same engine (same blocking semantics).


def _split_multi_waits(nc):
    for fn in nc.m.functions:
        for bb in fn.blocks:
            insts = bb.instructions
            new = []
            changed = False
            for inst in insts:
                si = inst.sync_info
                if si is not None and len(si.on_wait) > 1:
                    changed = True
                    waits = list(si.on_wait)
                    for w in waits[:-1]:
                        nop = mybir.InstNoOp(
                            name=f"I-wsplit-{nc.next_id()}", ins=[], outs=[]
                        )
                        nop.engine = inst.engine
                        nop.sync_info = mybir.SyncInfo(on_wait=[w], on_update=[])
                        nc.register_instruction(nop)
                        new.append(nop)
                    inst.sync_info = mybir.SyncInfo(
                        on_wait=[waits[-1]], on_update=list(si.on_update)
                    )
                new.append(inst)
            if changed:
                bb.instructions = new


# ---------------------------------------------------------------------------


def _fill_identity(nc, ident_ap, fill):
    """ident[x, y] = fill if x == y else 0."""
    nc.gpsimd.memset(ident_ap, 0.0)
    nc.gpsimd.affine_select(
        out=ident_ap,
        in_=ident_ap,
        compare_op=mybir.AluOpType.not_equal,
        fill=fill,
        base=0,
        pattern=[[-1, ident_ap.shape[1]]],
        channel_multiplier=1,
    )


def build_graph(expbt_schraudolph=False):
    """Build the per-core Bass graph (same graph SPMD on all 8 cores)."""
    nc = bass.Bass(dynamic_dma_scratch_size=65536)

    # --- DRAM parameters (per-core shards; names must match in_maps keys) ---
    p_qx = nc.declare_dram_parameter("q_x", [QS, C], FP32, isOutput=False)
    p_kvx = nc.declare_dram_parameter("kv_x", [K, C], FP32, isOutput=False)
    p_bias = nc.declare_dram_parameter("bias", [QS, K], FP32, isOutput=False)
    p_wq = nc.declare_dram_parameter("Wq", [C, C], FP32, isOutput=False)
    p_wk = nc.declare_dram_parameter("Wk", [C, C], FP32, isOutput=False)
    p_wv = nc.declare_dram_parameter("Wv", [C, C], FP32, isOutput=False)
    p_wo = nc.declare_dram_parameter("Wo", [C, C], FP32, isOutput=False)
    p_bo = nc.declare_dram_parameter("bo", [C], FP32, isOutput=False)
    p_wg = nc.declare_dram_parameter("Wg", [C, C], FP32, isOutput=False)
    p_bg = nc.declare_dram_parameter("bg", [C], FP32, isOutput=False)
    p_out = nc.declare_dram_parameter("out", [QS, C], FP32, isOutput=True)

    NKT = K // 128  # 16 key tiles
    NCT = C // 128  # 2 feature tiles
    NQT = QS // 128  # 4 query sub-tiles

    with tile_mod.TileContext(nc) as tc:
        with (
            tc.tile_pool(name="const", bufs=1) as constp,
            tc.tile_pool(name="persist", bufs=1) as persist,
            tc.tile_pool(name="dram", bufs=1, space="DRAM") as dramp,
        ):
            # ---- constants ----
            ident = constp.tile([128, 128], BF16, tag="ident")
            _fill_identity(nc, ident[:], 1.0)
            ident_s = constp.tile([128, 128], BF16, tag="ident_s")
            _fill_identity(nc, ident_s[:], float(SCALE))
            ones_mat = constp.tile([128, 32], BF16, tag="ones_mat")
            nc.gpsimd.memset(ones_mat[:], 1.0)
            ones_row = constp.tile([1, 128], BF16, tag="ones_row")
            nc.gpsimd.memset(ones_row[:], 1.0)

            bg_half = constp.tile([128, NCT], FP32, tag="bg_half")
            nc.gpsimd.dma_start(
                bg_half[:], p_bg[:].rearrange("(ct p) -> p ct", p=128)
            )
            nc.vector.tensor_scalar_mul(bg_half[:], bg_half[:], 0.5)
            bo_row = constp.tile([1, C], BF16, tag="bo_row")
            nc.gpsimd.dma_start(bo_row[:], p_bo[:].rearrange("(a c) -> a c", a=1))

            # ---- bf16 DRAM copies (cast during DMA), ordered for latency:
            # weights first (small, unblock PE), then the first bias chunk
            # and kv column-slab so wave 0's prerequisites land early.
            bias_bf = [
                dramp.tile([QS, 512], BF16, name=f"bias_bf{g}") for g in range(4)
            ]
            kv_bf = [
                dramp.tile([K, 128], BF16, name=f"kv_bf{ct}") for ct in range(2)
            ]
            qx_bf = dramp.tile([QS, C], BF16, name="qx_bf")
            nc.gpsimd.dma_start(bias_bf[0][:], p_bias[:, 0:512])
            nc.gpsimd.dma_start(kv_bf[0][:], p_kvx[:, 0:128])
            nc.gpsimd.dma_start(kv_bf[1][:], p_kvx[:, 128:256])
            nc.gpsimd.dma_start(qx_bf[:], p_qx[:])
            w_nat_tiles = {}
            for name, par in (
                ("Wq", p_wq), ("Wk", p_wk), ("Wv", p_wv), ("Wg", p_wg),
                ("Wo", p_wo),
            ):
                wn = persist.tile(
                    [128, NCT, C], BF16, tag=f"w_nat_{name}", name=f"w_nat_{name}"
                )
                nc.gpsimd.dma_start(
                    wn[:], par[:].rearrange("(jt p) c -> p jt c", p=128)
                )
                w_nat_tiles[name] = wn
            for g in range(1, 4):
                nc.gpsimd.dma_start(
                    bias_bf[g][:], p_bias[:, g * 512 : (g + 1) * 512]
                )

            # ---- transpose-loads (HWDGE xbar), split across both rings ----
            kvT = []
            for ct in range(NCT):
                sb = persist.tile([128, K], BF16, tag=f"kvT_{ct}", name=f"kvT_{ct}")
                nc.sync.dma_start(out=sb[:], in_=kv_bf[ct][:], transpose=True)
                kvT.append(sb)
            qxT = []
            for ct in range(NCT):
                sb = persist.tile([128, QS], BF16, tag=f"qxT_{ct}", name=f"qxT_{ct}")
                nc.sync.dma_start(
                    out=sb[:], in_=qx_bf[:, ct * 128 : (ct + 1) * 128],
                    transpose=True,
                )
                qxT.append(sb)

            expBT = [
                persist.tile([128, QS], BF16, tag=f"expBT{kt}", name=f"expBT{kt}")
                for kt in range(NKT)
            ]

            wt = {}
            kT, qT = [], []
            g_half = []

            with (
                tc.tile_pool(name="stage", bufs=2) as stage,
                tc.tile_pool(name="evp", bufs=4, space="PSUM") as evp,
            ):
                # biasT strips (alternating HWDGE rings) -> exp -> expBT
                for kt in range(NKT):
                    bT = stage.tile(
                        [128, QS], BF16, tag="biasT", bufs=4, name=f"biasT_{kt}"
                    )
                    nc.sync.dma_start(
                        out=bT[:],
                        in_=bias_bf[kt // 4][:, (kt % 4) * 128 : (kt % 4 + 1) * 128],
                        transpose=True,
                    )
                    if expbt_schraudolph:
                        nc.vector.tensor_scalar(
                            expBT[kt][:].bitcast(mybir.dt.int16),
                            bT[:],
                            184.6650558,
                            16256.035,
                            mybir.AluOpType.mult,
                            mybir.AluOpType.add,
                        )
                    else:
                        nc.scalar.activation(
                            expBT[kt][:], bT[:], mybir.ActivationFunctionType.Exp
                        )

                # ---- weight transposes (PE identity-matmuls) ----
                for name in ("Wq", "Wk", "Wv", "Wg", "Wo"):
                    w_nat = w_nat_tiles[name]
                    tiles = []
                    for ct in range(NCT):
                        ps = evp.tile([128, 512], FP32, tag="ev")
                        for jt in range(NCT):
                            nc.tensor.matmul(
                                ps[:, jt * 128 : (jt + 1) * 128],
                                w_nat[:, jt, ct * 128 : (ct + 1) * 128],
                                ident_s[:] if name == "Wq" else ident[:],
                                start=True,
                                stop=True,
                            )
                        sb = persist.tile([128, C], BF16, tag=f"wt_{name}_{ct}")
                        nc.vector.tensor_copy(sb[:], ps[:, :C])
                        tiles.append(sb)
                    wt[name] = tiles

                # ---- projections ----
                for jt in range(NCT):
                    ps = evp.tile([128, 512], FP32, tag="ev")
                    for ct in range(NCT):
                        nc.tensor.matmul(
                            ps[:],
                            wt["Wq"][ct][:, jt * 128 : (jt + 1) * 128],
                            qxT[ct][:],
                            start=(ct == 0),
                            stop=(ct == NCT - 1),
                        )
                    sb = persist.tile([128, QS], BF16, tag=f"qT_{jt}")
                    nc.vector.tensor_copy(sb[:], ps[:])
                    qT.append(sb)

                for jt in range(NCT):
                    sb = persist.tile([128, K], BF16, tag=f"kT_{jt}")
                    for tc_ in range(K // 512):
                        ps = evp.tile([128, 512], FP32, tag="ev")
                        for ct in range(NCT):
                            nc.tensor.matmul(
                                ps[:],
                                wt["Wk"][ct][:, jt * 128 : (jt + 1) * 128],
                                kvT[ct][:, tc_ * 512 : (tc_ + 1) * 512],
                                start=(ct == 0),
                                stop=(ct == NCT - 1),
                            )
                        nc.vector.tensor_copy(
                            sb[:, tc_ * 512 : (tc_ + 1) * 512], ps[:]
                        )
                    kT.append(sb)

                # v[kt]: [128, C] natural layout (partition = key token)
                v_sb = [
                    persist.tile([128, 4, C], BF16, tag=f"v_sb{g}", name=f"v_sb{g}")
                    for g in range(NKT // 4)
                ]
                for g in range(NKT // 4):
                    ps = evp.tile([128, 4, C], FP32, tag="ev4", bufs=2)
                    for i in range(4):
                        kt = g * 4 + i
                        for ct in range(NCT):
                            nc.tensor.matmul(
                                ps[:, i, :],
                                kvT[ct][:, kt * 128 : (kt + 1) * 128],
                                wt["Wv"][ct][:],
                                start=(ct == 0),
                                stop=(ct == NCT - 1),
                            )
                    nc.vector.tensor_copy(v_sb[g][:], ps[:])

                # gate: tanh(0.5*x + 0.5*bg); sigmoid(x+bg) = 0.5*tanh + 0.5
                for jt in range(NCT):
                    ps = evp.tile([128, 512], FP32, tag="ev")
                    for ct in range(NCT):
                        nc.tensor.matmul(
                            ps[:],
                            wt["Wg"][ct][:, jt * 128 : (jt + 1) * 128],
                            qxT[ct][:],
                            start=(ct == 0),
                            stop=(ct == NCT - 1),
                        )
                    th = persist.tile([128, QS], BF16, tag=f"gtanh_{jt}")
                    nc.scalar.activation(
                        th[:],
                        ps[:],
                        mybir.ActivationFunctionType.Tanh,
                        bias=bg_half[:, jt : jt + 1],
                        scale=0.5,
                    )
                    g_half.append(th)

            # ---- attention core ----
            ogT = []
            with (
                tc.tile_pool(name="acc", bufs=1, space="PSUM") as accp,
                tc.tile_pool(name="scores", bufs=1, space="PSUM") as scoresp,
                tc.tile_pool(name="expp", bufs=4) as expp,
            ):
                oT_ps = [
                    accp.tile([128, QS], FP32, tag=f"oT_{w}", name=f"oT_{w}")
                    for w in range(2)
                ]
                sums_ps = [
                    accp.tile([128, QS], FP32, tag=f"sums_{w}", name=f"sums_{w}")
                    for w in range(2)
                ]
                sc = scoresp.tile([128, 4, QS], FP32, tag="sc", name="sc")

                waves = [(kt, w) for kt in range(NKT) for w in range(2)]
                at_tiles = {}

                def emit_scores(i):
                    kt, w = waves[i]
                    ex = expp.tile([128, 4, QS], BF16, tag="ex", name=f"ex_{i}")
                    at = expp.tile([128, 4, QS], BF16, tag="at", name=f"at_{i}")
                    # two 8-tile half-grids (heads hb*2..hb*2+1), each
                    # followed by its exp half so the next wave's scores
                    # can start as soon as a half is consumed.
                    for hb in range(2):
                        for dd in range(4):
                            for hh in range(2):
                                s = 2 * hb + hh
                                ks = (dd + 2 * hh) % 4
                                nc.tensor.matmul(
                                    sc[32 * ks : 32 * (ks + 1), s, :],
                                    kT[w][
                                        32 * s : 32 * (s + 1),
                                        kt * 128 + 32 * ks : kt * 128
                                        + 32 * (ks + 1),
                                    ],
                                    qT[w][32 * s : 32 * (s + 1), :],
                                    start=True,
                                    stop=True,
                                    tile_position=(32 * s, 32 * ks),
                                )
                        nc.scalar.activation(
                            ex[:, 2 * hb : 2 * hb + 2, :],
                            sc[:, 2 * hb : 2 * hb + 2, :],
                            mybir.ActivationFunctionType.Exp,
                        )
                        nc.vector.tensor_tensor(
                            out=at[:, 2 * hb : 2 * hb + 2, :],
                            in0=ex[:, 2 * hb : 2 * hb + 2, :],
                            in1=expBT[kt][:]
                            .unsqueeze(1)
                            .broadcast_to((128, 2, QS)),
                            op=mybir.AluOpType.mult,
                        )
                    at_tiles[i] = at

                def emit_o_sums(i):
                    kt, w = waves[i]
                    at = at_tiles.pop(i)
                    first, last = kt == 0, kt == NKT - 1
                    for s in range(4):
                        nc.tensor.matmul(
                            oT_ps[w][32 * s : 32 * (s + 1), :],
                            v_sb[kt // 4][:, kt % 4, (4 * w + s) * D : (4 * w + s + 1) * D],
                            at[:, s, :],
                            start=first,
                            stop=last,
                            tile_position=(0, 32 * s),
                        )
                    for s in range(4):
                        nc.tensor.matmul(
                            sums_ps[w][32 * s : 32 * (s + 1), :],
                            ones_mat[:],
                            at[:, s, :],
                            start=first,
                            stop=last,
                            tile_position=(0, 32 * s),
                        )

                # depth-2 software pipeline
                for i in range(len(waves)):
                    emit_scores(i)
                    if i >= 2:
                        emit_o_sums(i - 2)
                emit_o_sums(len(waves) - 2)
                emit_o_sums(len(waves) - 1)

                # ---- normalize + gate:  og = oT * g * (1/Z) ----
                recipz = persist.tile([128, 2, QS], FP32, tag="recipz")
                for w in range(2):
                    nc.vector.reciprocal(recipz[:, w, :], sums_ps[w][:])
                for w in range(2):
                    zg = persist.tile([128, QS], BF16, tag=f"zg_{w}")
                    nc.vector.tensor_scalar(
                        zg[:],
                        g_half[w][:],
                        0.5,
                        0.5,
                        mybir.AluOpType.mult,
                        mybir.AluOpType.add,
                    )
                    nc.vector.tensor_mul(zg[:], zg[:], recipz[:, w, :])
                    og = persist.tile([128, QS], BF16, tag=f"ogT_{w}")
                    nc.vector.tensor_mul(og[:], oT_ps[w][:], zg[:])
                    ogT.append(og)

            # ---- output projection (natural layout) + bo ----
            out_sb = persist.tile([128, NQT, C], FP32, tag="out_sb")
            with tc.tile_pool(name="outp", bufs=2, space="PSUM") as outp:
                for qt in range(NQT):
                    ps = outp.tile([128, C], FP32, tag="outps")
                    for ct in range(NCT):
                        nc.tensor.matmul(
                            ps[:],
                            ogT[ct][:, qt * 128 : (qt + 1) * 128],
                            wt["Wo"][ct][:],
                            start=(ct == 0),
                            stop=False,
                        )
                    nc.tensor.matmul(
                        ps[:],
                        ones_row[:],
                        bo_row[:],
                        start=False,
                        stop=True,
                    )
                    nc.vector.tensor_copy(out_sb[:, qt, :], ps[:])

            nc.sync.dma_start(
                p_out[:].rearrange("(qt p) c -> p qt c", p=128), out_sb[:]
            )

    _split_multi_waits(nc)
    return nc


# ---------------------------------------------------------------------------


def _shard_inputs(inputs):
    """Full inputs -> per-core input maps."""
    in_maps = []
    for c in range(N_CORES):
        b, qc = divmod(c, 4)
        qs = qc * QS
        m = {
            "q_x": inputs["q_x"][b, qs : qs + QS, :],
            "kv_x": inputs["kv_x"][b],
            "bias": inputs["bias"][b, 0, qs : qs + QS, :],
            "Wq": inputs["Wq"],
            "Wk": inputs["Wk"],
            "Wv": inputs["Wv"],
            "Wo": inputs["Wo"],
            "bo": inputs["bo"],
            "Wg": inputs["Wg"],
            "bg": inputs["bg"],
        }
        m = {
            k: np.ascontiguousarray(np.asarray(v, dtype=np.float32))
            for k, v in m.items()
        }
        in_maps.append(m)
    return in_maps


def run(inputs, trace=False, tmpdir=None):
    """Run the kernel; returns (full_output, BassKernelResults)."""
    nc = build_graph()
    in_maps = _shard_inputs(inputs)
    res = run_bass_kernel_spmd(
        nc, in_maps, core_ids=list(range(N_CORES)), trace=trace, tmpdir=tmpdir
    )
    out = np.empty((B, Q, C), dtype=np.float32)
    for c in range(N_CORES):
        b, qc = divmod(c, 4)
        out[b, qc * QS : (qc + 1) * QS, :] = res.results[c]["out"]
    return out, res


def kernel(**inputs):
    out, _ = run(inputs, trace=False)
    return out


# revision 16
# speedup vs baseline: 1.1696x; 1.1696x over previous
"""Trainium2 Bass kernel for gated attention (nn_Attention_57475252355505).

Reference computation (per batch b):
    q = (q_x @ Wq.T) * 1/sqrt(32), split into H=8 heads of D=32
    k = kv_x @ Wk.T ; v = kv_x @ Wv.T
    a = softmax(q @ k.T + bias)           # bias broadcast over heads
    o = (a @ v) * sigmoid(q_x @ Wg.T + bg)
    out = o @ Wo.T + bo

Sharding: 8 cores, core c handles batch b = c//4 and query rows
[512*(c%4), 512*(c%4+1)).  kv_x/weights are replicated per batch group;
bias/q_x/output are disjoint.  No collectives needed.

Dataflow on each core is in "transposed space" ([feature, token] layouts)
so that every matmul contraction sits on the partition axis:
  - scores are computed as S^T [k, q] so softmax-over-k can use the
    matmul ones-trick for denominators, and the o-matmul needs no
    transposition of the (huge) attention-weight matrix.
  - bias^T is produced once with TensorE identity-matmuls and injected
    into the scores PSUM accumulation (so no elementwise bias pass).
  - the D=32 contractions are packed 4-per-PE-array with tile_position.
"""

import sys

sys.path.insert(0, "/opt/trn_rl_repo")

import numpy as np

import concourse.bass as bass
import concourse.mybir as mybir
import concourse.tile as tile_mod
from concourse.bass_utils import run_bass_kernel_spmd

# ---------------------------------------------------------------------------
# Problem constants (hardcoded per the harness contract).
B, Q, K, C, H, D = 2, 2048, 2048, 256, 8, 32
N_CORES = 8
QS = Q * B // N_CORES  # 512 query rows per core
SCALE = 1.0 / np.sqrt(np.float32(D))

FP32 = mybir.dt.float32
BF16 = mybir.dt.bfloat16

# ---------------------------------------------------------------------------
# This walrus build only accepts a single sync-wait per instruction; Tile's
# semaphore assignment batches several.  After tracing, hoist extra waits
# onto single-wait NOPs on the same engine (same blocking semantics).


def _split_multi_waits(nc):
    for fn in nc.m.functions:
        for bb in fn.blocks:
            insts = bb.instructions
            new = []
            changed = False
            for inst in insts:
                si = inst.sync_info
                if si is not None and len(si.on_wait) > 1:
                    changed = True
                    waits = list(si.on_wait)
                    for w in waits[:-1]:
                        nop = mybir.InstNoOp(
                            name=f"I-wsplit-{nc.next_id()}", ins=[], outs=[]
                        )
                        nop.engine = inst.engine
                        nop.sync_info = mybir.SyncInfo(on_wait=[w], on_update=[])
                        nc.register_instruction(nop)
                        new.append(nop)
                    inst.sync_info = mybir.SyncInfo(
                        on_wait=[waits[-1]], on_update=list(si.on_update)
                    )
                new.append(inst)
            if changed:
                bb.instructions = new


# ---------------------------------------------------------------------------


def _fill_identity(nc, ident_ap, fill):
    """ident[x, y] = fill if x == y else 0."""
    nc.gpsimd.memset(ident_ap, 0.0)
    nc.gpsimd.affine_select(
        out=ident_ap,
        in_=ident_ap,
        compare_op=mybir.AluOpType.not_equal,
        fill=fill,
        base=0,
        pattern=[[-1, ident_ap.shape[1]]],
        channel_multiplier=1,
    )


def build_graph(expbt_schraudolph=False):
    """Build the per-core Bass graph (same graph SPMD on all 8 cores)."""
    nc = bass.Bass(dynamic_dma_scratch_size=65536)

    # --- DRAM parameters (per-core shards; names must match in_maps keys) ---
    p_qx = nc.declare_dram_parameter("q_x", [QS, C], FP32, isOutput=False)
    p_kvx = nc.declare_dram_parameter("kv_x", [K, C], FP32, isOutput=False)
    p_bias = nc.declare_dram_parameter("bias", [QS, K], FP32, isOutput=False)
    p_wq = nc.declare_dram_parameter("Wq", [C, C], FP32, isOutput=False)
    p_wk = nc.declare_dram_parameter("Wk", [C, C], FP32, isOutput=False)
    p_wv = nc.declare_dram_parameter("Wv", [C, C], FP32, isOutput=False)
    p_wo = nc.declare_dram_parameter("Wo", [C, C], FP32, isOutput=False)
    p_bo = nc.declare_dram_parameter("bo", [C], FP32, isOutput=False)
    p_wg = nc.declare_dram_parameter("Wg", [C, C], FP32, isOutput=False)
    p_bg = nc.declare_dram_parameter("bg", [C], FP32, isOutput=False)
    p_out = nc.declare_dram_parameter("out", [QS, C], FP32, isOutput=True)

    NKT = K // 128  # 16 key tiles
    NCT = C // 128  # 2 feature tiles
    NQT = QS // 128  # 4 query sub-tiles

    with tile_mod.TileContext(nc) as tc:
        with (
            tc.tile_pool(name="const", bufs=1) as constp,
            tc.tile_pool(name="persist", bufs=1) as persist,
            tc.tile_pool(name="dram", bufs=1, space="DRAM") as dramp,
        ):
            # ---- constants ----
            ident = constp.tile([128, 128], BF16, tag="ident")
            _fill_identity(nc, ident[:], 1.0)
            ident_s = constp.tile([128, 128], BF16, tag="ident_s")
            _fill_identity(nc, ident_s[:], float(SCALE))
            ones_mat = constp.tile([128, 32], BF16, tag="ones_mat")
            nc.gpsimd.memset(ones_mat[:], 1.0)
            ones_row = constp.tile([1, 128], BF16, tag="ones_row")
            nc.gpsimd.memset(ones_row[:], 1.0)

            bg_half = constp.tile([128, NCT], FP32, tag="bg_half")
            nc.gpsimd.dma_start(
                bg_half[:], p_bg[:].rearrange("(ct p) -> p ct", p=128)
            )
            nc.vector.tensor_scalar_mul(bg_half[:], bg_half[:], 0.5)
            bo_row = constp.tile([1, C], BF16, tag="bo_row")
            nc.gpsimd.dma_start(bo_row[:], p_bo[:].rearrange("(a c) -> a c", a=1))

            # ---- bf16 DRAM copies (cast during DMA), ordered for latency:
            # weights first (small, unblock PE), then the first bias chunk
            # and kv column-slab so wave 0's prerequisites land early.
            # contiguous whole-tensor casts on SWDGE (few descriptors);
            # strided second hops go on HWDGE.
            w_bf = {}
            for name, par in (
                ("Wq", p_wq), ("Wk", p_wk), ("Wv", p_wv), ("Wg", p_wg),
                ("Wo", p_wo),
            ):
                wb = dramp.tile([C, C], BF16, name=f"w_bf_{name}")
                nc.gpsimd.dma_start(wb[:], par[:])
                w_bf[name] = wb
            bias_bf = dramp.tile([QS, K], BF16, name="bias_bf")
            nc.gpsimd.dma_start(bias_bf[:], p_bias[:])
            qx_bf = dramp.tile([QS, C], BF16, name="qx_bf")
            nc.gpsimd.dma_start(qx_bf[:], p_qx[:])
            kv_bf = dramp.tile([K, C], BF16, name="kv_bf")
            nc.gpsimd.dma_start(kv_bf[:], p_kvx[:])
            w_nat_tiles = {}
            for name in ("Wq", "Wk", "Wv", "Wg", "Wo"):
                wn = persist.tile(
                    [128, NCT, C], BF16, tag=f"w_nat_{name}", name=f"w_nat_{name}"
                )
                nc.sync.dma_start(
                    wn[:], w_bf[name][:].rearrange("(jt p) c -> p jt c", p=128)
                )
                w_nat_tiles[name] = wn

            # ---- transpose-loads (HWDGE xbar), split across both rings ----
            kvT = []
            for ct in range(NCT):
                sb = persist.tile([128, K], BF16, tag=f"kvT_{ct}", name=f"kvT_{ct}")
                nc.sync.dma_start(
                    out=sb[:], in_=kv_bf[:, ct * 128 : (ct + 1) * 128],
                    transpose=True,
                )
                kvT.append(sb)
            qxT = []
            for ct in range(NCT):
                sb = persist.tile([128, QS], BF16, tag=f"qxT_{ct}", name=f"qxT_{ct}")
                nc.sync.dma_start(
                    out=sb[:], in_=qx_bf[:, ct * 128 : (ct + 1) * 128],
                    transpose=True,
                )
                qxT.append(sb)

            expBT = [
                persist.tile([128, QS], BF16, tag=f"expBT{kt}", name=f"expBT{kt}")
                for kt in range(NKT)
            ]

            wt = {}
            kT, qT = [], []
            g_half = []

            with (
                tc.tile_pool(name="stage", bufs=2) as stage,
                tc.tile_pool(name="evp", bufs=4, space="PSUM") as evp,
            ):
                # biasT strips (alternating HWDGE rings) -> exp -> expBT
                for kt in range(NKT):
                    bT = stage.tile(
                        [128, QS], BF16, tag="biasT", bufs=4, name=f"biasT_{kt}"
                    )
                    nc.sync.dma_start(
                        out=bT[:],
                        in_=bias_bf[:, kt * 128 : (kt + 1) * 128],
                        transpose=True,
                    )
                    if expbt_schraudolph:
                        nc.vector.tensor_scalar(
                            expBT[kt][:].bitcast(mybir.dt.int16),
                            bT[:],
                            184.6650558,
                            16256.035,
                            mybir.AluOpType.mult,
                            mybir.AluOpType.add,
                        )
                    else:
                        nc.scalar.activation(
                            expBT[kt][:], bT[:], mybir.ActivationFunctionType.Exp
                        )

                # ---- weight transposes (PE identity-matmuls) ----
                for name in ("Wq", "Wk", "Wv", "Wg", "Wo"):
                    w_nat = w_nat_tiles[name]
                    tiles = []
                    for ct in range(NCT):
                        ps = evp.tile([128, 512], FP32, tag="ev")
                        for jt in range(NCT):
                            nc.tensor.matmul(
                                ps[:, jt * 128 : (jt + 1) * 128],
                                w_nat[:, jt, ct * 128 : (ct + 1) * 128],
                                ident_s[:] if name == "Wq" else ident[:],
                                start=True,
                                stop=True,
                            )
                        sb = persist.tile([128, C], BF16, tag=f"wt_{name}_{ct}")
                        nc.vector.tensor_copy(sb[:], ps[:, :C])
                        tiles.append(sb)
                    wt[name] = tiles

                # ---- projections ----
                for jt in range(NCT):
                    ps = evp.tile([128, 512], FP32, tag="ev")
                    for ct in range(NCT):
                        nc.tensor.matmul(
                            ps[:],
                            wt["Wq"][ct][:, jt * 128 : (jt + 1) * 128],
                            qxT[ct][:],
                            start=(ct == 0),
                            stop=(ct == NCT - 1),
                        )
                    sb = persist.tile([128, QS], BF16, tag=f"qT_{jt}")
                    nc.vector.tensor_copy(sb[:], ps[:])
                    qT.append(sb)

                for jt in range(NCT):
                    sb = persist.tile([128, K], BF16, tag=f"kT_{jt}")
                    for tc_ in range(K // 512):
                        ps = evp.tile([128, 512], FP32, tag="ev")
                        for ct in range(NCT):
                            nc.tensor.matmul(
                                ps[:],
                                wt["Wk"][ct][:, jt * 128 : (jt + 1) * 128],
                                kvT[ct][:, tc_ * 512 : (tc_ + 1) * 512],
                                start=(ct == 0),
                                stop=(ct == NCT - 1),
                            )
                        nc.vector.tensor_copy(
                            sb[:, tc_ * 512 : (tc_ + 1) * 512], ps[:]
                        )
                    kT.append(sb)

                # v[kt]: [128, C] natural layout (partition = key token)
                v_sb = [
                    persist.tile([128, 4, C], BF16, tag=f"v_sb{g}", name=f"v_sb{g}")
                    for g in range(NKT // 4)
                ]
                for g in range(NKT // 4):
                    ps = evp.tile([128, 4, C], FP32, tag="ev4", bufs=2)
                    for i in range(4):
                        kt = g * 4 + i
                        for ct in range(NCT):
                            nc.tensor.matmul(
                                ps[:, i, :],
                                kvT[ct][:, kt * 128 : (kt + 1) * 128],
                                wt["Wv"][ct][:],
                                start=(ct == 0),
                                stop=(ct == NCT - 1),
                            )
                    nc.vector.tensor_copy(v_sb[g][:], ps[:])

                # gate: tanh(0.5*x + 0.5*bg); sigmoid(x+bg) = 0.5*tanh + 0.5
                for jt in range(NCT):
                    ps = evp.tile([128, 512], FP32, tag="ev")
                    for ct in range(NCT):
                        nc.tensor.matmul(
                            ps[:],
                            wt["Wg"][ct][:, jt * 128 : (jt + 1) * 128],
                            qxT[ct][:],
                            start=(ct == 0),
                            stop=(ct == NCT - 1),
                        )
                    th = persist.tile([128, QS], BF16, tag=f"gtanh_{jt}")
                    nc.scalar.activation(
                        th[:],
                        ps[:],
                        mybir.ActivationFunctionType.Tanh,
                        bias=bg_half[:, jt : jt + 1],
                        scale=0.5,
                    )
                    g_half.append(th)

            # ---- attention core ----
            ogT = []
            with (
                tc.tile_pool(name="acc", bufs=1, space="PSUM") as accp,
                tc.tile_pool(name="scores", bufs=1, space="PSUM") as scoresp,
                tc.tile_pool(name="expp", bufs=4) as expp,
            ):
                oT_ps = [
                    accp.tile([128, QS], FP32, tag=f"oT_{w}", name=f"oT_{w}")
                    for w in range(2)
                ]
                sums_ps = [
                    accp.tile([128, QS], FP32, tag=f"sums_{w}", name=f"sums_{w}")
                    for w in range(2)
                ]
                sc = scoresp.tile([128, 4, QS], FP32, tag="sc", name="sc")

                waves = [(kt, w) for kt in range(NKT) for w in range(2)]
                at_tiles = {}

                def emit_scores(i):
                    kt, w = waves[i]
                    ex = expp.tile([128, 4, QS], BF16, tag="ex", name=f"ex_{i}")
                    at = expp.tile([128, 4, QS], BF16, tag="at", name=f"at_{i}")
                    # two 8-tile half-grids (heads hb*2..hb*2+1), each
                    # followed by its exp half so the next wave's scores
                    # can start as soon as a half is consumed.
                    for hb in range(2):
                        for dd in range(4):
                            for hh in range(2):
                                s = 2 * hb + hh
                                ks = (dd + 2 * hh) % 4
                                nc.tensor.matmul(
                                    sc[32 * ks : 32 * (ks + 1), s, :],
                                    kT[w][
                                        32 * s : 32 * (s + 1),
                                        kt * 128 + 32 * ks : kt * 128
                                        + 32 * (ks + 1),
                                    ],
                                    qT[w][32 * s : 32 * (s + 1), :],
                                    start=True,
                                    stop=True,
                                    tile_position=(32 * s, 32 * ks),
                                )
                        nc.scalar.activation(
                            ex[:, 2 * hb : 2 * hb + 2, :],
                            sc[:, 2 * hb : 2 * hb + 2, :],
                            mybir.ActivationFunctionType.Exp,
                        )
                        nc.vector.tensor_tensor(
                            out=at[:, 2 * hb : 2 * hb + 2, :],
                            in0=ex[:, 2 * hb : 2 * hb + 2, :],
                            in1=expBT[kt][:]
                            .unsqueeze(1)
                            .broadcast_to((128, 2, QS)),
                            op=mybir.AluOpType.mult,
                        )
                    at_tiles[i] = at

                def emit_o_sums(i):
                    kt, w = waves[i]
                    at = at_tiles.pop(i)
                    first, last = kt == 0, kt == NKT - 1
                    for s in range(4):
                        nc.tensor.matmul(
                            oT_ps[w][32 * s : 32 * (s + 1), :],
                            v_sb[kt // 4][:, kt % 4, (4 * w + s) * D : (4 * w + s + 1) * D],
                            at[:, s, :],
                            start=first,
                            stop=last,
                            tile_position=(0, 32 * s),
                        )
                    for s in range(4):
                        nc.tensor.matmul(
                            sums_ps[w][32 * s : 32 * (s + 1), :],
                            ones_mat[:],
                            at[:, s, :],
                            start=first,
                            stop=last,
                            tile_position=(0, 32 * s),
                        )

                # depth-2 software pipeline
                for i in range(len(waves)):
                    emit_scores(i)
                    if i >= 2:
                        emit_o_sums(i - 2)
                emit_o_sums(len(waves) - 2)
                emit_o_sums(len(waves) - 1)

                # ---- normalize + gate:  og = oT * g * (1/Z) ----
                recipz = persist.tile([128, 2, QS], FP32, tag="recipz")
                for w in range(2):
                    nc.vector.reciprocal(recipz[:, w, :], sums_ps[w][:])
                for w in range(2):
                    zg = persist.tile([128, QS], BF16, tag=f"zg_{w}")
                    nc.vector.tensor_scalar(
                        zg[:],
                        g_half[w][:],
                        0.5,
                        0.5,
                        mybir.AluOpType.mult,
                        mybir.AluOpType.add,
                    )
                    nc.vector.tensor_mul(zg[:], zg[:], recipz[:, w, :])
                    og = persist.tile([128, QS], BF16, tag=f"ogT_{w}")
                    nc.vector.tensor_mul(og[:], oT_ps[w][:], zg[:])
                    ogT.append(og)

            # ---- output projection (natural layout) + bo ----
            out_sb = persist.tile([128, NQT, C], FP32, tag="out_sb")
            with tc.tile_pool(name="outp", bufs=2, space="PSUM") as outp:
                for qt in range(NQT):
                    ps = outp.tile([128, C], FP32, tag="outps")
                    for ct in range(NCT):
                        nc.tensor.matmul(
                            ps[:],
                            ogT[ct][:, qt * 128 : (qt + 1) * 128],
                            wt["Wo"][ct][:],
                            start=(ct == 0),
                            stop=False,
                        )
                    nc.tensor.matmul(
                        ps[:],
                        ones_row[:],
                        bo_row[:],
                        start=False,
                        stop=True,
                    )
                    nc.vector.tensor_copy(out_sb[:, qt, :], ps[:])

            nc.sync.dma_start(
                p_out[:].rearrange("(qt p) c -> p qt c", p=128), out_sb[:]
            )

    _split_multi_waits(nc)
    return nc


# ---------------------------------------------------------------------------


def _shard_inputs(inputs):
    """Full inputs -> per-core input maps."""
    in_maps = []
    for c in range(N_CORES):
        b, qc = divmod(c, 4)
        qs = qc * QS
        m = {
            "q_x": inputs["q_x"][b, qs : qs + QS, :],
            "kv_x": inputs["kv_x"][b],
            "bias": inputs["bias"][b, 0, qs : qs + QS, :],
            "Wq": inputs["Wq"],
            "Wk": inputs["Wk"],
            "Wv": inputs["Wv"],
            "Wo": inputs["Wo"],
            "bo": inputs["bo"],
            "Wg": inputs["Wg"],
            "bg": inputs["bg"],
        }
        m = {
            k: np.ascontiguousarray(np.asarray(v, dtype=np.float32))
            for k, v in m.items()
        }
        in_maps.append(m)
    return in_maps


def run(inputs, trace=False, tmpdir=None):
    """Run the kernel; returns (full_output, BassKernelResults)."""
    nc = build_graph()
    in_maps = _shard_inputs(inputs)
    res = run_bass_kernel_spmd(
        nc, in_maps, core_ids=list(range(N_CORES)), trace=trace, tmpdir=tmpdir
    )
    out = np.empty((B, Q, C), dtype=np.float32)
    for c in range(N_CORES):
        b, qc = divmod(c, 4)
        out[b, qc * QS : (qc + 1) * QS, :] = res.results[c]["out"]
    return out, res


def kernel(**inputs):
    out, _ = run(inputs, trace=False)
    return out


# revision 25
# speedup vs baseline: 1.3723x; 1.1733x over previous
"""Trainium2 Bass kernel for gated attention (nn_Attention_57475252355505).

Reference computation (per batch b):
    q = (q_x @ Wq.T) * 1/sqrt(32), split into H=8 heads of D=32
    k = kv_x @ Wk.T ; v = kv_x @ Wv.T
    a = softmax(q @ k.T + bias)           # bias broadcast over heads
    o = (a @ v) * sigmoid(q_x @ Wg.T + bg)
    out = o @ Wo.T + bo

Sharding: 8 cores, core c handles batch b = c//4 and query rows
[512*(c%4), 512*(c%4+1)).  kv_x/weights are replicated per batch group;
bias/q_x/output are disjoint.  No collectives needed.

Dataflow on each core is in "transposed space" ([feature, token] layouts)
so that every matmul contraction sits on the partition axis:
  - scores are computed as S^T [k, q] so softmax-over-k can use the
    matmul ones-trick for denominators, and the o-matmul needs no
    transposition of the (huge) attention-weight matrix.
  - bias^T is produced once with TensorE identity-matmuls and injected
    into the scores PSUM accumulation (so no elementwise bias pass).
  - the D=32 contractions are packed 4-per-PE-array with tile_position.
"""

import sys

sys.path.insert(0, "/opt/trn_rl_repo")

import numpy as np

import concourse.bass as bass
import concourse.mybir as mybir
import concourse.tile as tile_mod
from concourse.bass_utils import run_bass_kernel_spmd

# ---------------------------------------------------------------------------
# Problem constants (hardcoded per the harness contract).
B, Q, K, C, H, D = 2, 2048, 2048, 256, 8, 32
N_CORES = 8
QS = Q * B // N_CORES  # 512 query rows per core
SCALE = 1.0 / np.sqrt(np.float32(D))

FP32 = mybir.dt.float32
BF16 = mybir.dt.bfloat16

# ---------------------------------------------------------------------------
# This walrus build only accepts a single sync-wait per instruction; Tile's
# semaphore assignment batches several.  After tracing, hoist extra waits
# onto single-wait NOPs on the same engine (same blocking semantics).


def _split_multi_waits(nc):
    for fn in nc.m.functions:
        for bb in fn.blocks:
            insts = bb.instructions
            new = []
            changed = False
            for inst in insts:
                si = inst.sync_info
                if si is not None and len(si.on_wait) > 1:
                    changed = True
                    waits = list(si.on_wait)
                    for w in waits[:-1]:
                        nop = mybir.InstNoOp(
                            name=f"I-wsplit-{nc.next_id()}", ins=[], outs=[]
                        )
                        nop.engine = inst.engine
                        nop.sync_info = mybir.SyncInfo(on_wait=[w], on_update=[])
                        nc.register_instruction(nop)
                        new.append(nop)
                    inst.sync_info = mybir.SyncInfo(
                        on_wait=[waits[-1]], on_update=list(si.on_update)
                    )
                new.append(inst)
            if changed:
                bb.instructions = new


# ---------------------------------------------------------------------------


def _fill_identity(nc, ident_ap, fill):
    """ident[x, y] = fill if x == y else 0."""
    nc.gpsimd.memset(ident_ap, 0.0)
    nc.gpsimd.affine_select(
        out=ident_ap,
        in_=ident_ap,
        compare_op=mybir.AluOpType.not_equal,
        fill=fill,
        base=0,
        pattern=[[-1, ident_ap.shape[1]]],
        channel_multiplier=1,
    )


def build_graph(expbt_schraudolph=False):
    """Build the per-core Bass graph (same graph SPMD on all 8 cores)."""
    nc = bass.Bass(dynamic_dma_scratch_size=65536)

    p_qx = nc.declare_dram_parameter("q_x", [QS, C], FP32, isOutput=False)
    p_kvx = nc.declare_dram_parameter("kv_x", [K, C], FP32, isOutput=False)
    p_bias = nc.declare_dram_parameter("bias", [QS, K], FP32, isOutput=False)
    p_wq = nc.declare_dram_parameter("Wq", [C, C], FP32, isOutput=False)
    p_wk = nc.declare_dram_parameter("Wk", [C, C], FP32, isOutput=False)
    p_wv = nc.declare_dram_parameter("Wv", [C, C], FP32, isOutput=False)
    p_wo = nc.declare_dram_parameter("Wo", [C, C], FP32, isOutput=False)
    p_bo = nc.declare_dram_parameter("bo", [C], FP32, isOutput=False)
    p_wg = nc.declare_dram_parameter("Wg", [C, C], FP32, isOutput=False)
    p_bg = nc.declare_dram_parameter("bg", [C], FP32, isOutput=False)
    p_out = nc.declare_dram_parameter("out", [QS, C], FP32, isOutput=True)

    NKT = K // 128
    NCT = C // 128
    NQT = QS // 128

    with tile_mod.TileContext(nc) as tc:
        with (
            tc.tile_pool(name="const", bufs=1) as constp,
            tc.tile_pool(name="persist", bufs=1) as persist,
            tc.tile_pool(name="dram", bufs=1, space="DRAM") as dramp,
        ):
            # ---- constants ----
            ident = constp.tile([128, 128], BF16, tag="ident")
            _fill_identity(nc, ident[:], 1.0)
            ident_s = constp.tile([128, 128], BF16, tag="ident_s")
            _fill_identity(nc, ident_s[:], float(SCALE))
            ones_row = constp.tile([1, 128], BF16, tag="ones_row")
            nc.gpsimd.memset(ones_row[:], 1.0)
            bg_half = constp.tile([128, NCT], FP32, tag="bg_half")
            nc.gpsimd.dma_start(
                bg_half[:], p_bg[:].rearrange("(ct p) -> p ct", p=128)
            )
            nc.vector.tensor_scalar_mul(bg_half[:], bg_half[:], 0.5)
            bo_row = constp.tile([1, C], BF16, tag="bo_row")
            nc.gpsimd.dma_start(bo_row[:], p_bo[:].rearrange("(a c) -> a c", a=1))

            # ---- SWDGE casts to bf16 DRAM scratch (contiguous-dst) ----
            w_bf = {}
            for name, par in (
                ("Wq", p_wq), ("Wk", p_wk), ("Wv", p_wv), ("Wg", p_wg),
                ("Wo", p_wo),
            ):
                wb = dramp.tile([C, C], BF16, name=f"w_bf_{name}")
                nc.gpsimd.dma_start(wb[:], par[:])
                w_bf[name] = wb
            kv_bf = dramp.tile([K, C], BF16, name="kv_bf")
            nc.gpsimd.dma_start(kv_bf[:], p_kvx[:])
            qx_bf = dramp.tile([QS, C], BF16, name="qx_bf")
            nc.gpsimd.dma_start(qx_bf[:], p_qx[:])
            bias_bf = [
                dramp.tile([QS, 512], BF16, name=f"bias_bf{g}") for g in range(4)
            ]
            for g in range(4):
                nc.gpsimd.dma_start(
                    bias_bf[g][:], p_bias[:, g * 512 : (g + 1) * 512]
                )

            # ---- HWDGE (sync ring): w_nat loads, then xbar transpose-loads
            w_nat_tiles = {}
            for name in ("Wq", "Wk", "Wv", "Wg", "Wo"):
                wn = persist.tile(
                    [128, NCT, C], BF16, tag=f"w_nat_{name}", name=f"w_nat_{name}"
                )
                nc.sync.dma_start(
                    wn[:], w_bf[name][:].rearrange("(jt p) c -> p jt c", p=128)
                )
                w_nat_tiles[name] = wn
            kvT = []
            for ct in range(NCT):
                sb = persist.tile([128, K], BF16, tag=f"kvT_{ct}", name=f"kvT_{ct}")
                nc.sync.dma_start(
                    out=sb[:], in_=kv_bf[:, ct * 128 : (ct + 1) * 128],
                    transpose=True,
                )
                kvT.append(sb)
            qxT = []
            for ct in range(NCT):
                sb = persist.tile([128, QS], BF16, tag=f"qxT_{ct}", name=f"qxT_{ct}")
                nc.sync.dma_start(
                    out=sb[:], in_=qx_bf[:, ct * 128 : (ct + 1) * 128],
                    transpose=True,
                )
                qxT.append(sb)

            expBT = [
                persist.tile([128, QS], BF16, tag=f"expBT{kt}", name=f"expBT{kt}")
                for kt in range(NKT)
            ]

            wt = {}
            kT, qT = [], []
            g_half = []
            # v with the softmax-denominator ones baked in: [v_h | ones32]
            aug_v = [
                persist.tile(
                    [128, 4, 8, 64], BF16, tag=f"aug_v{g}", name=f"aug_v{g}"
                )
                for g in range(NKT // 4)
            ]
            for g in range(NKT // 4):
                nc.gpsimd.memset(aug_v[g][:], 1.0)

            with (
                tc.tile_pool(name="stage", bufs=2) as stage,
                tc.tile_pool(name="evp", bufs=2, space="PSUM") as evp,
            ):
                # bias strips -> exp(bias^T)
                for kt in range(NKT):
                    bT = stage.tile(
                        [128, QS], BF16, tag="biasT", bufs=4, name=f"biasT_{kt}"
                    )
                    nc.sync.dma_start(
                        out=bT[:],
                        in_=bias_bf[kt // 4][:, (kt % 4) * 128 : (kt % 4 + 1) * 128],
                        transpose=True,
                    )
                    if expbt_schraudolph:
                        nc.vector.tensor_scalar(
                            expBT[kt][:].bitcast(mybir.dt.int16),
                            bT[:],
                            184.6650558,
                            16256.035,
                            mybir.AluOpType.mult,
                            mybir.AluOpType.add,
                        )
                    else:
                        nc.scalar.activation(
                            expBT[kt][:], bT[:], mybir.ActivationFunctionType.Exp
                        )

                # ---- weight transposes ----
                for name in ("Wq", "Wk", "Wv", "Wg", "Wo"):
                    w_nat = w_nat_tiles[name]
                    tiles = []
                    for ct in range(NCT):
                        ps = evp.tile([128, 512], FP32, tag="ev")
                        for jt in range(NCT):
                            nc.tensor.matmul(
                                ps[:, jt * 128 : (jt + 1) * 128],
                                w_nat[:, jt, ct * 128 : (ct + 1) * 128],
                                ident_s[:] if name == "Wq" else ident[:],
                                start=True,
                                stop=True,
                            )
                        sb = persist.tile([128, C], BF16, tag=f"wt_{name}_{ct}")
                        nc.vector.tensor_copy(sb[:], ps[:, :C])
                        tiles.append(sb)
                    wt[name] = tiles

                # ---- projections ----
                for jt in range(NCT):
                    ps = evp.tile([128, 512], FP32, tag="ev")
                    for ct in range(NCT):
                        nc.tensor.matmul(
                            ps[:],
                            wt["Wq"][ct][:, jt * 128 : (jt + 1) * 128],
                            qxT[ct][:],
                            start=(ct == 0),
                            stop=(ct == NCT - 1),
                        )
                    sb = persist.tile([128, QS], BF16, tag=f"qT_{jt}")
                    nc.vector.tensor_copy(sb[:], ps[:])
                    qT.append(sb)

                for jt in range(NCT):
                    sb = persist.tile([128, K], BF16, tag=f"kT_{jt}", name=f"kT_{jt}")
                    for tc_ in range(K // 512):
                        ps = evp.tile([128, 512], FP32, tag="ev")
                        for ct in range(NCT):
                            nc.tensor.matmul(
                                ps[:],
                                wt["Wk"][ct][:, jt * 128 : (jt + 1) * 128],
                                kvT[ct][:, tc_ * 512 : (tc_ + 1) * 512],
                                start=(ct == 0),
                                stop=(ct == NCT - 1),
                            )
                        nc.scalar.copy(sb[:, tc_ * 512 : (tc_ + 1) * 512], ps[:])
                    kT.append(sb)

                for g in range(NKT // 4):
                    ps4 = evp.tile([128, 4, C], FP32, tag="ev4", bufs=1)
                    for i in range(4):
                        for ct in range(NCT):
                            nc.tensor.matmul(
                                ps4[:, i, :],
                                kvT[ct][
                                    :, (g * 4 + i) * 128 : (g * 4 + i + 1) * 128
                                ],
                                wt["Wv"][ct][:],
                                start=(ct == 0),
                                stop=(ct == NCT - 1),
                            )
                    nc.vector.tensor_copy(
                        aug_v[g][:, :, :, 0:32],
                        ps4[:].rearrange("p i (h d) -> p i h d", h=8),
                    )

                # gate: tanh(0.5*x + 0.5*bg); sigmoid(x+bg) = 0.5*tanh + 0.5
                for jt in range(NCT):
                    ps = evp.tile([128, 512], FP32, tag="ev")
                    for ct in range(NCT):
                        nc.tensor.matmul(
                            ps[:],
                            wt["Wg"][ct][:, jt * 128 : (jt + 1) * 128],
                            qxT[ct][:],
                            start=(ct == 0),
                            stop=(ct == NCT - 1),
                        )
                    th = persist.tile([128, QS], BF16, tag=f"gtanh_{jt}")
                    nc.scalar.activation(
                        th[:],
                        ps[:],
                        mybir.ActivationFunctionType.Tanh,
                        bias=bg_half[:, jt : jt + 1],
                        scale=0.5,
                    )
                    g_half.append(th)

            # aug-row-layout gate (sigmoid affine applied later) and Wo^T:
            # bank b = 2w+p covers heads h0 = 4w+2p (rows 0-63) and h0+1
            # (rows 64-127); oT rows are 0-31 / 64-95.
            g_aug = [
                persist.tile([128, QS], BF16, tag=f"g_aug{b}", name=f"g_aug{b}")
                for b in range(4)
            ]
            woT_aug = [
                persist.tile([128, C], BF16, tag=f"woT_aug{b}", name=f"woT_aug{b}")
                for b in range(4)
            ]
            for b in range(4):
                w, p = b // 2, b % 2
                nc.gpsimd.memset(g_aug[b][:], 0.0)
                nc.gpsimd.memset(woT_aug[b][:], 0.0)
                for hh in range(2):
                    nc.sync.dma_start(
                        g_aug[b][64 * hh : 64 * hh + 32, :],
                        g_half[w][64 * p + 32 * hh : 64 * p + 32 * hh + 32, :],
                    )
                    nc.sync.dma_start(
                        woT_aug[b][64 * hh : 64 * hh + 32, :],
                        wt["Wo"][w][64 * p + 32 * hh : 64 * p + 32 * hh + 32, :],
                    )

            # ---- attention core ----
            og_aug = []
            with (
                tc.tile_pool(name="acc", bufs=1, space="PSUM") as accp,
                tc.tile_pool(name="scores", bufs=1, space="PSUM") as scoresp,
                tc.tile_pool(name="expp", bufs=4) as expp,
            ):
                aug_ps = [
                    accp.tile([128, QS], FP32, tag=f"aug_{b}", name=f"aug_{b}")
                    for b in range(4)
                ]
                sc = scoresp.tile([128, 4, QS], FP32, tag="sc", name="sc")

                waves = [(kt, w) for kt in range(NKT) for w in range(2)]
                at_tiles = {}

                def emit_scores(i):
                    kt, w = waves[i]
                    ex = expp.tile([128, 4, QS], BF16, tag="ex", name=f"ex_{i}")
                    at = expp.tile([128, 4, QS], BF16, tag="at", name=f"at_{i}")
                    for hb in range(2):
                        for dd in range(4):
                            for hh in range(2):
                                s = 2 * hb + hh
                                ks = (dd + 2 * hh) % 4
                                nc.tensor.matmul(
                                    sc[32 * ks : 32 * (ks + 1), s, :],
                                    kT[w][
                                        32 * s : 32 * (s + 1),
                                        kt * 128 + 32 * ks : kt * 128
                                        + 32 * (ks + 1),
                                    ],
                                    qT[w][32 * s : 32 * (s + 1), :],
                                    start=True,
                                    stop=True,
                                    tile_position=(32 * s, 32 * ks),
                                )
                        nc.scalar.activation(
                            ex[:, 2 * hb : 2 * hb + 2, :],
                            sc[:, 2 * hb : 2 * hb + 2, :],
                            mybir.ActivationFunctionType.Exp,
                        )
                        nc.vector.tensor_tensor(
                            out=at[:, 2 * hb : 2 * hb + 2, :],
                            in0=ex[:, 2 * hb : 2 * hb + 2, :],
                            in1=expBT[kt][:]
                            .unsqueeze(1)
                            .broadcast_to((128, 2, QS)),
                            op=mybir.AluOpType.mult,
                        )
                    at_tiles[i] = at

                def emit_o(i):
                    kt, w = waves[i]
                    at = at_tiles.pop(i)
                    first, last = kt == 0, kt == NKT - 1
                    for p in range(2):
                        for hh in range(2):
                            s = 2 * p + hh
                            nc.tensor.matmul(
                                aug_ps[2 * w + p][64 * hh : 64 * (hh + 1), :],
                                aug_v[kt // 4][:, kt % 4, 4 * w + s, :],
                                at[:, s, :],
                                start=first,
                                stop=last,
                                tile_position=(0, 64 * hh),
                                skip_group_check=True,
                            )

                for i in range(len(waves)):
                    if i >= 2:
                        emit_o(i - 2)
                    emit_scores(i)
                emit_o(len(waves) - 2)
                emit_o(len(waves) - 1)

                # ---- normalize + gate in aug layout ----
                # rows 32-63 / 96-127 of each bank hold Z replicated; shift
                # 1/Z up to the oT rows (zero elsewhere kills the Z rows).
                recipz = [
                    persist.tile(
                        [128, QS], FP32, tag=f"recipz{b}", name=f"recipz{b}"
                    )
                    for b in range(4)
                ]
                zrec = [
                    persist.tile([128, QS], BF16, tag=f"zrec{b}", name=f"zrec{b}")
                    for b in range(4)
                ]
                for b in range(4):
                    nc.gpsimd.memset(zrec[b][:], 0.0)
                    for hh in range(2):
                        nc.vector.reciprocal_approx_fast(
                            out=recipz[b][64 * hh + 32 : 64 * hh + 64, :],
                            in_=aug_ps[b][64 * hh + 32 : 64 * hh + 64, :],
                        )
                        nc.gpsimd.dma_start(
                            zrec[b][64 * hh : 64 * hh + 32, :],
                            recipz[b][64 * hh + 32 : 64 * hh + 64, :],
                        )
                for b in range(4):
                    w = b // 2
                    og = persist.tile([128, QS], BF16, tag=f"og{b}", name=f"og{b}")
                    # sigmoid = 0.5*tanh + 0.5 folded in:
                    # og = aug * (0.5*g_tanh + 0.5) * zrec
                    nc.vector.tensor_scalar(
                        og[:],
                        g_aug[b][:],
                        0.5,
                        0.5,
                        mybir.AluOpType.mult,
                        mybir.AluOpType.add,
                    )
                    nc.vector.tensor_mul(og[:], og[:], zrec[b][:])
                    nc.vector.tensor_mul(og[:], og[:], aug_ps[b][:])
                    og_aug.append(og)

            # ---- output projection + bo ----
            out_sb = persist.tile([128, NQT, C], FP32, tag="out_sb")
            with tc.tile_pool(name="outp", bufs=2, space="PSUM") as outp:
                for qt in range(NQT):
                    ps = outp.tile([128, C], FP32, tag="outps")
                    for b in range(4):
                        nc.tensor.matmul(
                            ps[:],
                            og_aug[b][:, qt * 128 : (qt + 1) * 128],
                            woT_aug[b][:],
                            start=(b == 0),
                            stop=False,
                        )
                    nc.tensor.matmul(
                        ps[:], ones_row[:], bo_row[:], start=False, stop=True
                    )
                    nc.vector.tensor_copy(out_sb[:, qt, :], ps[:])

            nc.sync.dma_start(
                p_out[:].rearrange("(qt p) c -> p qt c", p=128), out_sb[:]
            )

    _split_multi_waits(nc)
    return nc


# ---------------------------------------------------------------------------


def _shard_inputs(inputs):
    """Full inputs -> per-core input maps."""
    in_maps = []
    for c in range(N_CORES):
        b, qc = divmod(c, 4)
        qs = qc * QS
        m = {
            "q_x": inputs["q_x"][b, qs : qs + QS, :],
            "kv_x": inputs["kv_x"][b],
            "bias": inputs["bias"][b, 0, qs : qs + QS, :],
            "Wq": inputs["Wq"],
            "Wk": inputs["Wk"],
            "Wv": inputs["Wv"],
            "Wo": inputs["Wo"],
            "bo": inputs["bo"],
            "Wg": inputs["Wg"],
            "bg": inputs["bg"],
        }
        m = {
            k: np.ascontiguousarray(np.asarray(v, dtype=np.float32))
            for k, v in m.items()
        }
        in_maps.append(m)
    return in_maps


def run(inputs, trace=False, tmpdir=None):
    """Run the kernel; returns (full_output, BassKernelResults)."""
    nc = build_graph()
    in_maps = _shard_inputs(inputs)
    res = run_bass_kernel_spmd(
        nc, in_maps, core_ids=list(range(N_CORES)), trace=trace, tmpdir=tmpdir
    )
    out = np.empty((B, Q, C), dtype=np.float32)
    for c in range(N_CORES):
        b, qc = divmod(c, 4)
        out[b, qc * QS : (qc + 1) * QS, :] = res.results[c]["out"]
    return out, res


def kernel(**inputs):
    out, _ = run(inputs, trace=False)
    return out


# revision 26
# speedup vs baseline: 1.4407x; 1.0498x over previous
"""Trainium2 Bass kernel for gated attention (nn_Attention_57475252355505).

Reference computation (per batch b):
    q = (q_x @ Wq.T) * 1/sqrt(32), split into H=8 heads of D=32
    k = kv_x @ Wk.T ; v = kv_x @ Wv.T
    a = softmax(q @ k.T + bias)           # bias broadcast over heads
    o = (a @ v) * sigmoid(q_x @ Wg.T + bg)
    out = o @ Wo.T + bo

Sharding: 8 cores, core c handles batch b = c//4 and query rows
[512*(c%4), 512*(c%4+1)).  kv_x/weights are replicated per batch group;
bias/q_x/output are disjoint.  No collectives needed.

Dataflow on each core is in "transposed space" ([feature, token] layouts)
so that every matmul contraction sits on the partition axis:
  - scores are computed as S^T [k, q] so softmax-over-k can use the
    matmul ones-trick for denominators, and the o-matmul needs no
    transposition of the (huge) attention-weight matrix.
  - bias^T is produced once with TensorE identity-matmuls and injected
    into the scores PSUM accumulation (so no elementwise bias pass).
  - the D=32 contractions are packed 4-per-PE-array with tile_position.
"""

import sys

sys.path.insert(0, "/opt/trn_rl_repo")

import numpy as np

import concourse.bass as bass
import concourse.mybir as mybir
import concourse.tile as tile_mod
from concourse.bass_utils import run_bass_kernel_spmd

# ---------------------------------------------------------------------------
# Problem constants (hardcoded per the harness contract).
B, Q, K, C, H, D = 2, 2048, 2048, 256, 8, 32
N_CORES = 8
QS = Q * B // N_CORES  # 512 query rows per core
SCALE = 1.0 / np.sqrt(np.float32(D))

FP32 = mybir.dt.float32
BF16 = mybir.dt.bfloat16

# ---------------------------------------------------------------------------
# This walrus build only accepts a single sync-wait per instruction; Tile's
# semaphore assignment batches several.  After tracing, hoist extra waits
# onto single-wait NOPs on the same engine (same blocking semantics).


def _split_multi_waits(nc):
    for fn in nc.m.functions:
        for bb in fn.blocks:
            insts = bb.instructions
            new = []
            changed = False
            for inst in insts:
                si = inst.sync_info
                if si is not None and len(si.on_wait) > 1:
                    changed = True
                    waits = list(si.on_wait)
                    for w in waits[:-1]:
                        nop = mybir.InstNoOp(
                            name=f"I-wsplit-{nc.next_id()}", ins=[], outs=[]
                        )
                        nop.engine = inst.engine
                        nop.sync_info = mybir.SyncInfo(on_wait=[w], on_update=[])
                        nc.register_instruction(nop)
                        new.append(nop)
                    inst.sync_info = mybir.SyncInfo(
                        on_wait=[waits[-1]], on_update=list(si.on_update)
                    )
                new.append(inst)
            if changed:
                bb.instructions = new


# ---------------------------------------------------------------------------


def _fill_identity(nc, ident_ap, fill):
    """ident[x, y] = fill if x == y else 0."""
    nc.gpsimd.memset(ident_ap, 0.0)
    nc.gpsimd.affine_select(
        out=ident_ap,
        in_=ident_ap,
        compare_op=mybir.AluOpType.not_equal,
        fill=fill,
        base=0,
        pattern=[[-1, ident_ap.shape[1]]],
        channel_multiplier=1,
    )


def build_graph(expbt_schraudolph=False):
    """Build the per-core Bass graph (same graph SPMD on all 8 cores)."""
    nc = bass.Bass(dynamic_dma_scratch_size=65536)

    p_qx = nc.declare_dram_parameter("q_x", [QS, C], FP32, isOutput=False)
    p_kvx = nc.declare_dram_parameter("kv_x", [K, C], FP32, isOutput=False)
    p_bias = nc.declare_dram_parameter("bias", [QS, K], FP32, isOutput=False)
    p_wq = nc.declare_dram_parameter("Wq", [C, C], FP32, isOutput=False)
    p_wk = nc.declare_dram_parameter("Wk", [C, C], FP32, isOutput=False)
    p_wv = nc.declare_dram_parameter("Wv", [C, C], FP32, isOutput=False)
    p_wo = nc.declare_dram_parameter("Wo", [C, C], FP32, isOutput=False)
    p_bo = nc.declare_dram_parameter("bo", [C], FP32, isOutput=False)
    p_wg = nc.declare_dram_parameter("Wg", [C, C], FP32, isOutput=False)
    p_bg = nc.declare_dram_parameter("bg", [C], FP32, isOutput=False)
    p_out = nc.declare_dram_parameter("out", [QS, C], FP32, isOutput=True)

    NKT = K // 128
    NCT = C // 128
    NQT = QS // 128

    with tile_mod.TileContext(nc) as tc:
        with (
            tc.tile_pool(name="const", bufs=1) as constp,
            tc.tile_pool(name="persist", bufs=1) as persist,
            tc.tile_pool(name="dram", bufs=1, space="DRAM") as dramp,
        ):
            # ---- constants ----
            ident = constp.tile([128, 128], BF16, tag="ident")
            _fill_identity(nc, ident[:], 1.0)
            ident_s = constp.tile([128, 128], BF16, tag="ident_s")
            _fill_identity(nc, ident_s[:], float(SCALE))
            ones_row = constp.tile([1, 128], BF16, tag="ones_row")
            nc.gpsimd.memset(ones_row[:], 1.0)
            bg_half = constp.tile([128, NCT], FP32, tag="bg_half")
            nc.gpsimd.dma_start(
                bg_half[:], p_bg[:].rearrange("(ct p) -> p ct", p=128)
            )
            nc.vector.tensor_scalar_mul(bg_half[:], bg_half[:], 0.5)
            bo_row = constp.tile([1, C], BF16, tag="bo_row")
            nc.gpsimd.dma_start(bo_row[:], p_bo[:].rearrange("(a c) -> a c", a=1))

            # ---- SWDGE casts to bf16 DRAM scratch (contiguous-dst) ----
            w_bf = {}
            for name, par in (
                ("Wq", p_wq), ("Wk", p_wk), ("Wv", p_wv), ("Wg", p_wg),
                ("Wo", p_wo),
            ):
                wb = dramp.tile([C, C], BF16, name=f"w_bf_{name}")
                nc.gpsimd.dma_start(wb[:], par[:])
                w_bf[name] = wb
            kv_bf = dramp.tile([K, C], BF16, name="kv_bf")
            nc.gpsimd.dma_start(kv_bf[:], p_kvx[:])
            qx_bf = dramp.tile([QS, C], BF16, name="qx_bf")
            nc.gpsimd.dma_start(qx_bf[:], p_qx[:])
            bias_bf = [
                dramp.tile([QS, 512], BF16, name=f"bias_bf{g}") for g in range(4)
            ]
            for g in range(4):
                nc.gpsimd.dma_start(
                    bias_bf[g][:], p_bias[:, g * 512 : (g + 1) * 512]
                )

            # ---- HWDGE (sync ring): w_nat loads, then xbar transpose-loads
            w_nat_tiles = {}
            for name in ("Wq", "Wk", "Wv", "Wg", "Wo"):
                wn = persist.tile(
                    [128, NCT, C], BF16, tag=f"w_nat_{name}", name=f"w_nat_{name}"
                )
                nc.sync.dma_start(
                    wn[:], w_bf[name][:].rearrange("(jt p) c -> p jt c", p=128)
                )
                w_nat_tiles[name] = wn
            kvT = []
            for ct in range(NCT):
                sb = persist.tile([128, K], BF16, tag=f"kvT_{ct}", name=f"kvT_{ct}")
                nc.sync.dma_start(
                    out=sb[:], in_=kv_bf[:, ct * 128 : (ct + 1) * 128],
                    transpose=True,
                )
                kvT.append(sb)
            qxT = []
            for ct in range(NCT):
                sb = persist.tile([128, QS], BF16, tag=f"qxT_{ct}", name=f"qxT_{ct}")
                nc.sync.dma_start(
                    out=sb[:], in_=qx_bf[:, ct * 128 : (ct + 1) * 128],
                    transpose=True,
                )
                qxT.append(sb)

            expBT = [
                persist.tile([128, QS], BF16, tag=f"expBT{kt}", name=f"expBT{kt}")
                for kt in range(NKT)
            ]

            wt = {}
            kT, qT = [], []
            g_half = []
            # v with the softmax-denominator ones baked in: [v_h | ones32]
            aug_v = [
                persist.tile(
                    [128, 4, 8, 64], BF16, tag=f"aug_v{g}", name=f"aug_v{g}"
                )
                for g in range(NKT // 4)
            ]
            for g in range(NKT // 4):
                nc.gpsimd.memset(aug_v[g][:], 1.0)

            with (
                tc.tile_pool(name="stage", bufs=2) as stage,
                tc.tile_pool(name="evp", bufs=4, space="PSUM") as evp,
            ):
                # bias strips -> exp(bias^T)
                for kt in range(NKT):
                    bT = stage.tile(
                        [128, QS], BF16, tag="biasT", bufs=4, name=f"biasT_{kt}"
                    )
                    nc.sync.dma_start(
                        out=bT[:],
                        in_=bias_bf[kt // 4][:, (kt % 4) * 128 : (kt % 4 + 1) * 128],
                        transpose=True,
                    )
                    if expbt_schraudolph:
                        nc.vector.tensor_scalar(
                            expBT[kt][:].bitcast(mybir.dt.int16),
                            bT[:],
                            184.6650558,
                            16256.035,
                            mybir.AluOpType.mult,
                            mybir.AluOpType.add,
                        )
                    else:
                        nc.scalar.activation(
                            expBT[kt][:], bT[:], mybir.ActivationFunctionType.Exp
                        )

                # ---- weight transposes ----
                for name in ("Wq", "Wk", "Wv", "Wg", "Wo"):
                    w_nat = w_nat_tiles[name]
                    tiles = []
                    for ct in range(NCT):
                        ps = evp.tile([128, 512], FP32, tag="ev")
                        for jt in range(NCT):
                            nc.tensor.matmul(
                                ps[:, jt * 128 : (jt + 1) * 128],
                                w_nat[:, jt, ct * 128 : (ct + 1) * 128],
                                ident_s[:] if name == "Wq" else ident[:],
                                start=True,
                                stop=True,
                            )
                        sb = persist.tile([128, C], BF16, tag=f"wt_{name}_{ct}")
                        nc.vector.tensor_copy(sb[:], ps[:, :C])
                        tiles.append(sb)
                    wt[name] = tiles

                # ---- projections ----
                for jt in range(NCT):
                    ps = evp.tile([128, 512], FP32, tag="ev")
                    for ct in range(NCT):
                        nc.tensor.matmul(
                            ps[:],
                            wt["Wq"][ct][:, jt * 128 : (jt + 1) * 128],
                            qxT[ct][:],
                            start=(ct == 0),
                            stop=(ct == NCT - 1),
                        )
                    sb = persist.tile([128, QS], BF16, tag=f"qT_{jt}")
                    nc.vector.tensor_copy(sb[:], ps[:])
                    qT.append(sb)

                for jt in range(NCT):
                    sb = persist.tile([128, K], BF16, tag=f"kT_{jt}", name=f"kT_{jt}")
                    for tc_ in range(K // 512):
                        ps = evp.tile([128, 512], FP32, tag="ev")
                        for ct in range(NCT):
                            nc.tensor.matmul(
                                ps[:],
                                wt["Wk"][ct][:, jt * 128 : (jt + 1) * 128],
                                kvT[ct][:, tc_ * 512 : (tc_ + 1) * 512],
                                start=(ct == 0),
                                stop=(ct == NCT - 1),
                            )
                        nc.scalar.copy(sb[:, tc_ * 512 : (tc_ + 1) * 512], ps[:])
                    kT.append(sb)

                for g in range(NKT // 4):
                    ps4 = evp.tile([128, 4, C], FP32, tag="ev4", bufs=2)
                    for i in range(4):
                        for ct in range(NCT):
                            nc.tensor.matmul(
                                ps4[:, i, :],
                                kvT[ct][
                                    :, (g * 4 + i) * 128 : (g * 4 + i + 1) * 128
                                ],
                                wt["Wv"][ct][:],
                                start=(ct == 0),
                                stop=(ct == NCT - 1),
                            )
                    nc.vector.tensor_copy(
                        aug_v[g][:, :, :, 0:32],
                        ps4[:].rearrange("p i (h d) -> p i h d", h=8),
                    )

                # gate: tanh(0.5*x + 0.5*bg); sigmoid(x+bg) = 0.5*tanh + 0.5
                for jt in range(NCT):
                    ps = evp.tile([128, 512], FP32, tag="ev")
                    for ct in range(NCT):
                        nc.tensor.matmul(
                            ps[:],
                            wt["Wg"][ct][:, jt * 128 : (jt + 1) * 128],
                            qxT[ct][:],
                            start=(ct == 0),
                            stop=(ct == NCT - 1),
                        )
                    th = persist.tile([128, QS], BF16, tag=f"gtanh_{jt}")
                    nc.scalar.activation(
                        th[:],
                        ps[:],
                        mybir.ActivationFunctionType.Tanh,
                        bias=bg_half[:, jt : jt + 1],
                        scale=0.5,
                    )
                    g_half.append(th)

            # aug-row-layout gate (sigmoid affine applied later) and Wo^T:
            # bank b = 2w+p covers heads h0 = 4w+2p (rows 0-63) and h0+1
            # (rows 64-127); oT rows are 0-31 / 64-95.
            g_aug = [
                persist.tile([128, QS], BF16, tag=f"g_aug{b}", name=f"g_aug{b}")
                for b in range(4)
            ]
            woT_aug = [
                persist.tile([128, C], BF16, tag=f"woT_aug{b}", name=f"woT_aug{b}")
                for b in range(4)
            ]
            for b in range(4):
                w, p = b // 2, b % 2
                nc.gpsimd.memset(g_aug[b][:], 0.0)
                nc.gpsimd.memset(woT_aug[b][:], 0.0)
                for hh in range(2):
                    nc.sync.dma_start(
                        g_aug[b][64 * hh : 64 * hh + 32, :],
                        g_half[w][64 * p + 32 * hh : 64 * p + 32 * hh + 32, :],
                    )
                    nc.sync.dma_start(
                        woT_aug[b][64 * hh : 64 * hh + 32, :],
                        wt["Wo"][w][64 * p + 32 * hh : 64 * p + 32 * hh + 32, :],
                    )

            # ---- attention core ----
            og_aug = []
            with (
                tc.tile_pool(name="acc", bufs=1, space="PSUM") as accp,
                tc.tile_pool(name="scores", bufs=1, space="PSUM") as scoresp,
                tc.tile_pool(name="expp", bufs=4) as expp,
            ):
                aug_ps = [
                    accp.tile([128, QS], FP32, tag=f"aug_{b}", name=f"aug_{b}")
                    for b in range(4)
                ]
                sc = scoresp.tile([128, 4, QS], FP32, tag="sc", name="sc")

                waves = [(kt, w) for kt in range(NKT) for w in range(2)]
                at_tiles = {}

                def emit_scores(i):
                    kt, w = waves[i]
                    ex = expp.tile([128, 4, QS], BF16, tag="ex", name=f"ex_{i}")
                    at = expp.tile([128, 4, QS], BF16, tag="at", name=f"at_{i}")
                    for hb in range(2):
                        for dd in range(4):
                            for hh in range(2):
                                s = 2 * hb + hh
                                ks = (dd + 2 * hh) % 4
                                nc.tensor.matmul(
                                    sc[32 * ks : 32 * (ks + 1), s, :],
                                    kT[w][
                                        32 * s : 32 * (s + 1),
                                        kt * 128 + 32 * ks : kt * 128
                                        + 32 * (ks + 1),
                                    ],
                                    qT[w][32 * s : 32 * (s + 1), :],
                                    start=True,
                                    stop=True,
                                    tile_position=(32 * s, 32 * ks),
                                )
                        nc.scalar.activation(
                            ex[:, 2 * hb : 2 * hb + 2, :],
                            sc[:, 2 * hb : 2 * hb + 2, :],
                            mybir.ActivationFunctionType.Exp,
                        )
                        nc.vector.tensor_tensor(
                            out=at[:, 2 * hb : 2 * hb + 2, :],
                            in0=ex[:, 2 * hb : 2 * hb + 2, :],
                            in1=expBT[kt][:]
                            .unsqueeze(1)
                            .broadcast_to((128, 2, QS)),
                            op=mybir.AluOpType.mult,
                        )
                    at_tiles[i] = at

                def emit_o(i):
                    kt, w = waves[i]
                    at = at_tiles.pop(i)
                    first, last = kt == 0, kt == NKT - 1
                    for p in range(2):
                        for hh in range(2):
                            s = 2 * p + hh
                            nc.tensor.matmul(
                                aug_ps[2 * w + p][64 * hh : 64 * (hh + 1), :],
                                aug_v[kt // 4][:, kt % 4, 4 * w + s, :],
                                at[:, s, :],
                                start=first,
                                stop=last,
                                tile_position=(0, 64 * hh),
                                skip_group_check=True,
                            )

                for i in range(len(waves)):
                    if i >= 2:
                        emit_o(i - 2)
                    emit_scores(i)
                emit_o(len(waves) - 2)
                emit_o(len(waves) - 1)

                # ---- normalize + gate in aug layout ----
                # rows 32-63 / 96-127 of each bank hold Z replicated; shift
                # 1/Z up to the oT rows (zero elsewhere kills the Z rows).
                recipz = [
                    persist.tile(
                        [128, QS], FP32, tag=f"recipz{b}", name=f"recipz{b}"
                    )
                    for b in range(4)
                ]
                zrec = [
                    persist.tile([128, QS], BF16, tag=f"zrec{b}", name=f"zrec{b}")
                    for b in range(4)
                ]
                for b in range(4):
                    nc.gpsimd.memset(zrec[b][:], 0.0)
                    for hh in range(2):
                        nc.vector.reciprocal_approx_fast(
                            out=recipz[b][64 * hh + 32 : 64 * hh + 64, :],
                            in_=aug_ps[b][64 * hh + 32 : 64 * hh + 64, :],
                        )
                        nc.gpsimd.dma_start(
                            zrec[b][64 * hh : 64 * hh + 32, :],
                            recipz[b][64 * hh + 32 : 64 * hh + 64, :],
                        )
                for b in range(4):
                    w = b // 2
                    og = persist.tile([128, QS], BF16, tag=f"og{b}", name=f"og{b}")
                    # sigmoid = 0.5*tanh + 0.5 folded in:
                    # og = aug * (0.5*g_tanh + 0.5) * zrec
                    nc.vector.tensor_scalar(
                        og[:],
                        g_aug[b][:],
                        0.5,
                        0.5,
                        mybir.AluOpType.mult,
                        mybir.AluOpType.add,
                    )
                    nc.vector.tensor_mul(og[:], og[:], zrec[b][:])
                    nc.vector.tensor_mul(og[:], og[:], aug_ps[b][:])
                    og_aug.append(og)

            # ---- output projection + bo ----
            out_sb = persist.tile([128, NQT, C], FP32, tag="out_sb")
            with tc.tile_pool(name="outp", bufs=2, space="PSUM") as outp:
                for qt in range(NQT):
                    ps = outp.tile([128, C], FP32, tag="outps")
                    for b in range(4):
                        nc.tensor.matmul(
                            ps[:],
                            og_aug[b][:, qt * 128 : (qt + 1) * 128],
                            woT_aug[b][:],
                            start=(b == 0),
                            stop=False,
                        )
                    nc.tensor.matmul(
                        ps[:], ones_row[:], bo_row[:], start=False, stop=True
                    )
                    nc.vector.tensor_copy(out_sb[:, qt, :], ps[:])

            nc.sync.dma_start(
                p_out[:].rearrange("(qt p) c -> p qt c", p=128), out_sb[:]
            )

    _split_multi_waits(nc)
    return nc


# ---------------------------------------------------------------------------


def _shard_inputs(inputs):
    """Full inputs -> per-core input maps."""
    in_maps = []
    for c in range(N_CORES):
        b, qc = divmod(c, 4)
        qs = qc * QS
        m = {
            "q_x": inputs["q_x"][b, qs : qs + QS, :],
            "kv_x": inputs["kv_x"][b],
            "bias": inputs["bias"][b, 0, qs : qs + QS, :],
            "Wq": inputs["Wq"],
            "Wk": inputs["Wk"],
            "Wv": inputs["Wv"],
            "Wo": inputs["Wo"],
            "bo": inputs["bo"],
            "Wg": inputs["Wg"],
            "bg": inputs["bg"],
        }
        m = {
            k: np.ascontiguousarray(np.asarray(v, dtype=np.float32))
            for k, v in m.items()
        }
        in_maps.append(m)
    return in_maps


def run(inputs, trace=False, tmpdir=None):
    """Run the kernel; returns (full_output, BassKernelResults)."""
    nc = build_graph()
    in_maps = _shard_inputs(inputs)
    res = run_bass_kernel_spmd(
        nc, in_maps, core_ids=list(range(N_CORES)), trace=trace, tmpdir=tmpdir
    )
    out = np.empty((B, Q, C), dtype=np.float32)
    for c in range(N_CORES):
        b, qc = divmod(c, 4)
        out[b, qc * QS : (qc + 1) * QS, :] = res.results[c]["out"]
    return out, res


def kernel(**inputs):
    out, _ = run(inputs, trace=False)
    return out
